# revision 1
# baseline (speedup 1.0000x reference)
"""Trainium2 Bass kernel for nn_AttentionHead (B=8, S=2048, E=1024, Dh=64).

Sharding: data-parallel over batch B across the 8 NeuronCores (one batch
element per core); W/b replicated; results gathered on host.

Per-core computation in transposed orientation (scores^T[k,q]):
  qkv = x @ W + b  (1/sqrt(Dh) folded into W_q,b_q)
  scores^T = kT.T @ qT                  (PE, f32r, into a 2-bank PSUM pair)
  u = exp(scores^T)                     (ACT, one instr per k-chunk PAIR,
                                         PSUM -> SBUF bf16)
  u = where(mask^T, 1, u)               (DVE copy_predicated per pair;
                                         mask^T lives only in PSUM: the int32
                                         mask is viewed as bf16 via bitcast
                                         (low halfword 0/1 = 0.0/denormal),
                                         PE-transposed at 1 cycle/row, and the
                                         PSUM tile is re-viewed as int16 for
                                         the predicate - exact 0/1 semantics,
                                         no conversion or slab-copy passes)
  [num^T; Z] = [v | 1]^T @ u            (PE accumulate over k-chunks; ones
                                         column yields the softmax denominator)
  out = (num * (1/Z))^T                 (ACT numz copy, PE transpose, DVE
                                         recip, ACT per-partition scale)

DMA stream (the roofline resource, ~332 B/ns serialized): W, x (8 MiB),
then all mask tiles (16 MiB) in attend order on the in-order SP queue so
nothing preempts mask bytes; output stores issue after all mask DMAs.
Attend q-blocks are [384 x4, 256 x2]: wide early to amortize ACT init
overheads, narrow late so the critical path after the last mask bytes is
short. qkv runs as two stacked accumulations per e-chunk with stationaries
[W_q|W_k] and [W_k|W_v] (k computed twice) so q lands on lanes 0-63 and v
on lanes 64-127 lane-aligned for their consumers (16 not 24 matmuls per
s-block); v1 transposes are batched after the qkv stream and quad-packed.

HW constraints honored (BIR verifier): GPSIMD never touches PSUM, f32r
matmul inputs are produced as f32r (real rounded identity tile; x
transposes stay f32), copy_predicated masks are integer-typed, PE
Ldweights stationaries are float-typed.

Mask tiles stream as column-quarter DMAs (free: the stream rate is
per-byte) so each kc-pair unblocks as soon as its columns land; qkv
matmuls run in column halves so q/k/v finish closer behind the x stream.

Cost-model simulated span ~85.6 us/core (baseline 111.9): DMA-stream-bound
to ~75.8 us (x+mask at 332 B/ns, gap-free), with the attend pipeline
(ACT exp-pairs / DVE copy_predicated / PE scores+av+mask-transposes
co-paced at ~1 us per k-chunk pair) trailing each mask quarter's arrival,
plus a ~4 us last-block tail. Max rel err vs fp32 reference ~4.6e-3
(bf16 u/v1 rounding; f32r elsewhere).
"""

import os
import sys

sys.path.insert(0, "/opt/trn_rl_repo")

import numpy as np

import concourse.bass as bass
import concourse.tile as tile
from concourse import bacc, mybir
from concourse.masks import make_identity
from concourse.bass_utils import run_bass_kernel_spmd

F32 = mybir.dt.float32
F32R = mybir.dt.float32r
BF16 = mybir.dt.bfloat16
I32 = mybir.dt.int32
I16 = mybir.dt.int16

B, S, E, DH = 8, 2048, 1024, 64
N_CORES = 8
SCALE = 1.0 / 8.0  # 1/sqrt(DH)

MM_DT = F32R

EC = E // 128     # 8   e-chunks
SC = S // 128     # 16  s-chunks (k-chunks)
QB = S // 512     # 4   x s-blocks
KC = SC

# attend q-blocks: narrower late so the critical path after the last mask
# bytes land is short; 384 mid-size balances ACT init overhead vs DMA pacing
BLOCKS = [(0, 384), (384, 384), (768, 384), (1152, 384), (1536, 256), (1792, 256)]

AF = mybir.ActivationFunctionType
OP = mybir.AluOpType


def build(nc: bass.Bass):
    x_in = nc.dram_tensor("x", [S, E], F32, kind="ExternalInput")
    m_in = nc.dram_tensor("mask", [S, S], I32, kind="ExternalInput")
    w_in = nc.dram_tensor("W", [E, 3 * DH], F32, kind="ExternalInput")
    b_in = nc.dram_tensor("b", [3 * DH], F32, kind="ExternalInput")
    o_out = nc.dram_tensor("out", [S, DH], F32, kind="ExternalOutput")

    trace_sim = bool(os.environ.get("TRN_TRACE_SIM"))
    with tile.TileContext(nc, trace_sim=trace_sim) as tc:
        with (
            tc.tile_pool(name="persist", bufs=1) as persist,
            tc.tile_pool(name="small", bufs=1) as small,
        ):
            # ---- constants / weights -------------------------------------
            ident = persist.tile([128, 128], F32)
            make_identity(nc, ident)
            ident_bf = persist.tile([128, 128], BF16)
            nc.vector.tensor_copy(ident_bf[:], ident[:])
            ident_r = persist.tile([128, 128], F32R)
            nc.vector.tensor_copy(ident_r[:], ident[:])
            ones_bf = persist.tile([128, 2, 512], BF16)
            nc.vector.memset(ones_bf[:], 1.0)

            w_raw = small.tile([128, EC, 3 * DH], F32)
            nc.gpsimd.dma_start(w_raw[:], w_in.rearrange("(o p) d -> p o d", p=128))
            # stacked stationaries: wst1 = [W_q*scale | W_k], wst2 = [W_k | W_v]
            wst1 = persist.tile([128, EC, 128], MM_DT)
            wst2 = persist.tile([128, EC, 128], MM_DT)
            nc.vector.tensor_scalar_mul(wst1[:, :, 0:64], w_raw[:, :, 0:DH], SCALE)
            nc.vector.tensor_copy(wst1[:, :, 64:128], w_raw[:, :, DH : 2 * DH])
            nc.scalar.copy(wst2[:, :, 0:64], w_raw[:, :, DH : 2 * DH])
            nc.scalar.copy(wst2[:, :, 64:128], w_raw[:, :, 2 * DH : 3 * DH])

            b_q_raw = small.tile([64, 1], F32)
            nc.gpsimd.dma_start(b_q_raw[:], b_in[0:64].unsqueeze(-1))
            b_q = persist.tile([64, 1], F32)
            nc.vector.tensor_scalar_mul(b_q[:], b_q_raw[:], SCALE)
            b_k = persist.tile([64, 1], F32)
            nc.gpsimd.dma_start(b_k[:], b_in[64:128].unsqueeze(-1))
            b_v128 = persist.tile([128, 1], F32)  # v bias on lanes 64-127
            nc.gpsimd.dma_start(b_v128[64:128, :], b_in[128:192].unsqueeze(-1))

            # warm the ACT exp table early so the first real exp doesn't pay
            # the 1.3us table load
            warm = small.tile([1, 1], F32)
            nc.vector.memset(warm[:], 0.0)
            warm_o = small.tile([1, 1], BF16)
            nc.scalar.activation(warm_o[:], warm[:], AF.Exp)

            # persistent activations: qv rows 0-63 = q^T, rows 64-127 = v^T
            qv = persist.tile([128, S], MM_DT)
            kT = persist.tile([64, S], MM_DT)
            v1 = persist.tile([128, SC, DH + 1], BF16)  # v natural + ones col
            ones_col = small.tile([128, 1], F32)
            nc.vector.memset(ones_col[:], 1.0)
            for c in range(SC):
                nc.vector.tensor_copy(v1[:, c, DH : DH + 1], ones_col[:])

            # mask staging opened alongside phase-1 pools so its SBUF space
            # never aliases x tiles (aliasing stalls the DMA stream)
            from contextlib import ExitStack

            mask_ctx = ExitStack()
            p_m = mask_ctx.enter_context(tc.tile_pool(name="mstage", bufs=7))

            # ---- phase 1: x -> x^T -> q/k/v ------------------------------
            with (
                tc.tile_pool(name="xnat", bufs=2) as p_xnat,
                tc.tile_pool(name="xT", bufs=2) as p_xT,
                tc.tile_pool(name="ps_t", bufs=4, space="PSUM") as ps_t,
                tc.tile_pool(name="ps_mm", bufs=4, space="PSUM") as ps_mm,
            ):
                for nt in range(QB):
                    sl = slice(nt * 512, (nt + 1) * 512)
                    x_T = p_xT.tile([128, EC, 512], MM_DT)
                    x_nat4 = p_xnat.tile([128, 4, E], F32)
                    for h in range(2):
                        nc.sync.dma_start(
                            x_nat4[:, h * 2 : (h + 1) * 2, :],
                            x_in[nt * 512 + h * 256 : nt * 512 + (h + 1) * 256, :]
                            .rearrange("(c p) e -> p c e", p=128),
                        )
                    # group-major so the first 4 copies complete e-chunks 0-3
                    # and the qkv accumulation can start early
                    for g in range(2):
                        for c4 in range(4):
                            pst = ps_t.tile([128, 512], F32, name="pst")
                            for j4 in range(4):
                                j = g * 4 + j4
                                nc.tensor.transpose(
                                    pst[:, j4 * 128 : (j4 + 1) * 128],
                                    x_nat4[:, c4, j * 128 : (j + 1) * 128],
                                    ident[:],
                                )
                            dst = x_T[:, g * 4 : (g + 1) * 4, c4 * 128 : (c4 + 1) * 128]
                            src = pst[:].rearrange("p (j f) -> p j f", j=4)
                            if (g * 4 + c4) % 2 == 0:
                                nc.vector.tensor_copy(dst, src)
                            else:
                                nc.scalar.copy(dst, src)  # ACT

                    ps1 = ps_mm.tile([128, 512], F32, name="psmm")
                    ps2 = ps_mm.tile([128, 512], F32, name="psmm")
                    for hh in range(2):
                        cs = slice(hh * 256, (hh + 1) * 256)
                        gsl = slice(nt * 512 + hh * 256, nt * 512 + (hh + 1) * 256)
                        for j in range(EC):
                            nc.tensor.matmul(
                                ps1[:, cs], wst1[:, j, :], x_T[:, j, cs],
                                start=(j == 0), stop=(j == EC - 1),
                            )
                        for j in range(EC):
                            nc.tensor.matmul(
                                ps2[:, cs], wst2[:, j, :], x_T[:, j, cs],
                                start=(j == 0), stop=(j == EC - 1),
                            )
                        nc.vector.tensor_scalar_add(
                            qv[0:64, gsl], ps1[0:64, cs], b_q[:]
                        )
                        nc.vector.tensor_scalar_add(
                            kT[:, gsl], ps2[0:64, cs], b_k[:]
                        )
                        nc.vector.tensor_scalar_add(
                            qv[64:128, gsl], ps2[64:128, cs], b_v128[64:128, :]
                        )

                # v natural (+ones col), batched after the qkv stream so the
                # per-block PE queue never stalls on the v-bias chain; quad-
                # packed so the psum->sbuf copies amortize engine init
                for c4 in range(SC // 4):
                    psv = ps_t.tile([128, 512], F32R, name="pst")
                    for j in range(4):
                        c = c4 * 4 + j
                        nc.tensor.transpose(
                            psv[:, j * 64 : (j + 1) * 64],
                            qv[64:128, c * 128 : (c + 1) * 128],
                            ident_r[64:128, 64:128],
                        )
                    nc.scalar.copy(
                        v1[:, c4 * 4 : (c4 + 1) * 4, 0:DH],
                        psv[:, 0:256].rearrange("p (j f) -> p j f", j=4),
                    )


            # ---- mask DMA: all tiles, SP program order after x -----------
            with mask_ctx:
                m_tiles = {}
                for mc in range(SC):
                    m_i32 = p_m.tile([128, S], I32)
                    # column-quarter DMAs: the stream rate is per-byte so
                    # splitting is free (HWDGE issue still fits under the
                    # stream), and early kc-pairs unblock as their columns
                    # land instead of waiting for the tile's last bytes
                    for h in range(4):
                        nc.sync.dma_start(
                            m_i32[:, h * 512 : (h + 1) * 512],
                            m_in[mc * 128 : (mc + 1) * 128,
                                 h * 512 : (h + 1) * 512],
                        )
                    m_tiles[mc] = m_i32

                # ---- phase 2: attention ----------------------------------
                with (
                    tc.tile_pool(name="u", bufs=6) as p_u,
                    tc.tile_pool(name="nz", bufs=3) as p_nz,
                    tc.tile_pool(name="osb", bufs=3) as p_o,
                    tc.tile_pool(name="ps_s", bufs=2, space="PSUM") as ps_s,
                    tc.tile_pool(name="ps_tm", bufs=2, space="PSUM") as ps_tm,
                    tc.tile_pool(name="ps_o", bufs=1, space="PSUM") as ps_o,
                    tc.tile_pool(name="ps_t2", bufs=1, space="PSUM") as ps_t2,
                ):
                    for bi, (q0, width) in enumerate(BLOCKS):
                        nmc = width // 128
                        mviews = [
                            m_tiles[q0 // 128 + mc][:].bitcast(BF16)
                            for mc in range(nmc)
                        ]
                        po_f = ps_o.tile([DH + 1, 512], F32, name="po")
                        po = po_f[:, :width]
                        # software-pipelined: transposes+scores of pair p
                        # are emitted before exp/cp/av of pair p-1 so the
                        # in-order PE queue hides the cross-engine latency
                        stage = []
                        for kp in range(KC // 2):
                            pstm_f = ps_tm.tile([128, 2, 512], BF16, name="pstm")
                            upair_f = p_u.tile([128, 2, 512], BF16, tag="u")
                            pss_f = ps_s.tile([128, 2, 512], F32, name="pss")
                            for j in range(2):
                                kc = kp * 2 + j
                                pstm = pstm_f[:, j, :width]
                                for mc in range(nmc):
                                    nc.tensor.matmul(
                                        pstm[:, mc * 128 : (mc + 1) * 128],
                                        mviews[mc][:, 2 * kc * 128 : 2 * (kc + 1) * 128 : 2],
                                        ident_bf[:],
                                        is_transpose=True,
                                    )
                                nc.tensor.matmul(
                                    pss_f[:, j, :width],
                                    kT[:, kc * 128 : (kc + 1) * 128],
                                    qv[0:64, q0 : q0 + width],
                                    start=True,
                                    stop=True,
                                )
                            stage.append((kp, pstm_f, upair_f, pss_f))
                            if len(stage) == 2 or kp == KC // 2 - 1:
                                for kp2, pstm2, upair2, pss2 in (
                                    list(stage) if kp == KC // 2 - 1 else stage[:1]
                                ):
                                    nc.scalar.activation(
                                        upair2[:, :, :width],
                                        pss2[:, :, :width], AF.Exp,
                                    )
                                    nc.vector.copy_predicated(
                                        upair2[:, :, :width],
                                        pstm2[:, :, :width].bitcast(I16),
                                        ones_bf[:, :, :width],
                                    )
                                    for j in range(2):
                                        kc = kp2 * 2 + j
                                        nc.tensor.matmul(
                                            po[:],
                                            v1[:, kc, :],
                                            upair2[:, j, :width],
                                            start=(kc == 0),
                                            stop=(kc == KC - 1),
                                        )
                                    stage.remove((kp2, pstm2, upair2, pss2))

                        numz_f = p_nz.tile([DH + 1, 512], F32, tag="nz")
                        numz = numz_f[:, :width]
                        nc.scalar.copy(numz[:], po[:])
                        o_sbn_f = p_o.tile([128, 4, DH], F32, tag="osb")
                        o_sbn = o_sbn_f[:, :nmc, :]
                        pt2 = ps_t2.tile([128, 4, DH + 1], F32, name="pt2")
                        for i in range(nmc):
                            nc.tensor.transpose(
                                pt2[:, i, :],
                                numz[:, i * 128 : (i + 1) * 128],
                                ident[0 : DH + 1, 0 : DH + 1],
                            )
                        r_cols = p_o.tile([128, 4], F32, tag="rcol")
                        nc.vector.reciprocal(
                            r_cols[:, 0:nmc], pt2[:, 0:nmc, DH]
                        )
                        for i in range(nmc):
                            nc.scalar.activation(
                                o_sbn[:, i, :], pt2[:, i, 0:DH], AF.Copy,
                                scale=r_cols[:, i : i + 1],
                            )
                        nc.sync.dma_start(
                            o_out[q0 : q0 + width, :].rearrange(
                                "(i p) d -> p i d", p=128
                            ),
                            o_sbn[:],
                        )

    nc.finalize()
    return nc


_CACHED_NC = None


def _get_nc():
    global _CACHED_NC
    if _CACHED_NC is None:
        _CACHED_NC = build(bacc.Bacc())
    return _CACHED_NC


def kernel(x, mask, W, b, _trace=False, _tmpdir=None):
    """Full inputs in, full output out. Shards batch across 8 neuron cores."""
    x = np.ascontiguousarray(x, dtype=np.float32)
    mask = np.ascontiguousarray(mask, dtype=np.int32)
    W = np.ascontiguousarray(W, dtype=np.float32)
    b = np.ascontiguousarray(b, dtype=np.float32)
    assert x.shape == (B, S, E) and mask.shape == (B, S, S)

    nc = _get_nc()
    in_maps = [
        {"x": x[c], "mask": mask[c], "W": W, "b": b} for c in range(N_CORES)
    ]
    res = run_bass_kernel_spmd(
        nc, in_maps, list(range(N_CORES)), trace=_trace, tmpdir=_tmpdir
    )
    out = np.stack([res.results[c]["out"] for c in range(N_CORES)])
    if _trace:
        return out, res
    return out



# revision 9
# speedup vs baseline: 1.0237x; 1.0237x over previous
"""Trainium2 Bass kernel for nn_AttentionHead (B=8, S=2048, E=1024, Dh=64).

Sharding: data-parallel over batch B across the 8 NeuronCores (one batch
element per core); W/b replicated; results gathered on host.

Per-core computation in transposed orientation (scores^T[k,q]):
  qkv = x @ W + b  (1/sqrt(Dh) folded into W_q,b_q)
  scores^T = kT.T @ qT                  (PE, f32r, into a 2-bank PSUM pair)
  u = exp(scores^T)                     (ACT, one instr per k-chunk PAIR,
                                         PSUM -> SBUF bf16)
  u = where(mask^T, 1, u)               (DVE copy_predicated per pair;
                                         mask^T lives only in PSUM via
                                         bf16-bitcast PE transposes, re-viewed
                                         as int16 for the predicate)
  [num^T; Z] = [v | 1]^T @ u            (PE accumulate over k-chunks)
  out = (num * (1/Z))^T                 (ACT numz copy, PE transpose, DVE
                                         recip, ACT per-partition scale)

DMA: three parallel queues (SP HWDGE, ACT HWDGE, Pool SWDGE). In the cost
model each queue's transfers serialize at ~332 B/ns but different queues
overlap, so the 24.5 MiB of input streams in ~31 us instead of ~78 us.
  SP:   x half-blocks 0a..3a (4 x 1 MiB), then mask tiles 0,3,6,9,12,15
  ACT:  x half-blocks 0b..3b,             then mask tiles 2,5,8,11,14
  Pool: W, biases,                        then mask tiles 1,4,7,10,13,
        then per-block output stores
x transposes run at f32r (1.5 PE cycles/row instead of 2.0 for f32).
The span is then compute-bound: PE ~67 us busy (x-transpose 10.2, qkv 13.7,
scores 13.7, mask-transpose 13.7, av 13.7), DVE ~60, ACT ~57.
"""

import os
import sys

sys.path.insert(0, "/opt/trn_rl_repo")

import numpy as np

import concourse.bass as bass
import concourse.tile as tile
from concourse import bacc, mybir
from concourse.masks import make_identity
from concourse.bass_utils import run_bass_kernel_spmd

F32 = mybir.dt.float32
F32R = mybir.dt.float32r
BF16 = mybir.dt.bfloat16
I32 = mybir.dt.int32
I16 = mybir.dt.int16

B, S, E, DH = 8, 2048, 1024, 64
N_CORES = 8
SCALE = 1.0 / 8.0  # 1/sqrt(DH)

MM_DT = F32R

EC = E // 128     # 8   e-chunks
SC = S // 128     # 16  s-chunks (k-chunks)
QB = S // 512     # 4   x s-blocks
KC = SC

BLOCKS = [(0, 384), (384, 384), (768, 384), (1152, 384), (1536, 256), (1792, 256)]

AF = mybir.ActivationFunctionType
OP = mybir.AluOpType

# DMA transfer time occupies the issuing engine in the cost model, so bulk
# streams go on SP (no compute) and Pool (nearly idle); ACT carries only the
# small early W/bias loads its own wst/bias consumers need.
MASK_ENG = ["sync", "gpsimd"]


def build(nc: bass.Bass):
    x_in = nc.dram_tensor("x", [S, E], F32, kind="ExternalInput")
    m_in = nc.dram_tensor("mask", [S, S], I32, kind="ExternalInput")
    w_in = nc.dram_tensor("W", [E, 3 * DH], F32, kind="ExternalInput")
    b_in = nc.dram_tensor("b", [3 * DH], F32, kind="ExternalInput")
    o_out = nc.dram_tensor("out", [S, DH], F32, kind="ExternalOutput")

    trace_sim = bool(os.environ.get("TRN_TRACE_SIM"))
    with tile.TileContext(nc, trace_sim=trace_sim) as tc:
        with (
            tc.tile_pool(name="persist", bufs=1) as persist,
            tc.tile_pool(name="small", bufs=1) as small,
        ):
            # ---- constants / weights (Pool queue) ------------------------
            ident = persist.tile([128, 128], F32)
            make_identity(nc, ident)
            ident_bf = persist.tile([128, 128], BF16)
            nc.vector.tensor_copy(ident_bf[:], ident[:])
            ident_r = persist.tile([128, 128], F32R)
            nc.vector.tensor_copy(ident_r[:], ident[:])
            ones_bf = persist.tile([128, 2, 512], BF16)
            nc.vector.memset(ones_bf[:], 1.0)

            w_raw = small.tile([128, EC, 3 * DH], F32)
            nc.scalar.dma_start(w_raw[:], w_in.rearrange("(o p) d -> p o d", p=128))
            # stacked stationaries: wst1 = [W_q*scale | W_k], wst2 = [W_k | W_v]
            wst1 = persist.tile([128, EC, 128], MM_DT)
            wst2 = persist.tile([128, EC, 128], MM_DT)
            nc.vector.tensor_scalar_mul(wst1[:, :, 0:64], w_raw[:, :, 0:DH], SCALE)
            nc.vector.tensor_copy(wst1[:, :, 64:128], w_raw[:, :, DH : 2 * DH])
            nc.scalar.copy(wst2[:, :, 0:64], w_raw[:, :, DH : 2 * DH])
            nc.scalar.copy(wst2[:, :, 64:128], w_raw[:, :, 2 * DH : 3 * DH])

            b_q_raw = small.tile([64, 1], F32)
            nc.scalar.dma_start(b_q_raw[:], b_in[0:64].unsqueeze(-1))
            b_q = persist.tile([64, 1], F32)
            nc.vector.tensor_scalar_mul(b_q[:], b_q_raw[:], SCALE)
            b_k = persist.tile([64, 1], F32)
            nc.scalar.dma_start(b_k[:], b_in[64:128].unsqueeze(-1))
            b_v128 = persist.tile([128, 1], F32)  # v bias on lanes 64-127
            nc.scalar.dma_start(b_v128[64:128, :], b_in[128:192].unsqueeze(-1))

            # warm the ACT exp table early
            warm = small.tile([1, 1], F32)
            nc.vector.memset(warm[:], 0.0)
            warm_o = small.tile([1, 1], BF16)
            nc.scalar.activation(warm_o[:], warm[:], AF.Exp)

            # persistent activations: qv rows 0-63 = q^T, rows 64-127 = v^T
            qv = persist.tile([128, S], MM_DT)
            kT = persist.tile([64, S], MM_DT)
            v1 = persist.tile([128, SC, DH + 1], BF16)  # v natural + ones col
            ones_col = small.tile([128, 1], F32)
            nc.vector.memset(ones_col[:], 1.0)
            for c in range(SC):
                nc.vector.tensor_copy(v1[:, c, DH : DH + 1], ones_col[:])

            from contextlib import ExitStack

            mask_ctx = ExitStack()
            # 9 rotating slots: tile n+9 reuses tile n's space, whose consumer
            # block finishes well before tile n+9 is needed (no deadlock: the
            # attend consumes tiles strictly in order)
            p_m = mask_ctx.enter_context(tc.tile_pool(name="mstage", bufs=9))

            # ---- phase 1: x -> x^T -> q/k/v ------------------------------
            with (
                tc.tile_pool(name="xnat", bufs=2) as p_xnat,
                tc.tile_pool(name="xT", bufs=2) as p_xT,
                tc.tile_pool(name="ps_t", bufs=4, space="PSUM") as ps_t,
                tc.tile_pool(name="ps_mm", bufs=4, space="PSUM") as ps_mm,
            ):
                x_nats = []
                for nt in range(QB):
                    # x half-blocks: "a" half on SP, "b" half on Pool
                    x_nat4 = p_xnat.tile([128, 4, E], F32R)
                    for h, eng in ((0, nc.sync), (1, nc.gpsimd)):
                        eng.dma_start(
                            x_nat4[:, h * 2 : (h + 1) * 2, :],
                            x_in[nt * 512 + h * 256 : nt * 512 + (h + 1) * 256, :]
                            .rearrange("(c p) e -> p c e", p=128)
                            .bitcast(F32R),
                        )
                    x_nats.append(x_nat4)

                # mask tile DMAs, round-robin over the three queues, emitted
                # after x so each queue drains x first (program order)
                m_tiles = {}
                for mc in range(SC):
                    m_i32 = p_m.tile([128, S], I32)
                    eng = getattr(nc, MASK_ENG[mc % 2])
                    eng.dma_start(m_i32[:], m_in[mc * 128 : (mc + 1) * 128, :])
                    m_tiles[mc] = m_i32

                for nt in range(QB):
                    x_nat4 = x_nats[nt]
                    x_T = p_xT.tile([128, EC, 512], MM_DT)
                    # group-major so the first 4 copies complete e-chunks 0-3
                    for g in range(2):
                        for c4 in range(4):
                            pst = ps_t.tile([128, 512], F32R, name="pst")
                            for j4 in range(4):
                                j = g * 4 + j4
                                nc.tensor.transpose(
                                    pst[:, j4 * 128 : (j4 + 1) * 128],
                                    x_nat4[:, c4, j * 128 : (j + 1) * 128],
                                    ident_r[:],
                                )
                            dst = x_T[:, g * 4 : (g + 1) * 4, c4 * 128 : (c4 + 1) * 128]
                            src = pst[:].rearrange("p (j f) -> p j f", j=4)
                            if (g * 4 + c4) % 2 == 0:
                                nc.vector.tensor_copy(dst, src)
                            else:
                                nc.scalar.copy(dst, src)  # ACT

                    ps1 = ps_mm.tile([128, 512], F32, name="psmm")
                    ps2 = ps_mm.tile([128, 512], F32, name="psmm")
                    for hh in range(2):
                        cs = slice(hh * 256, (hh + 1) * 256)
                        gsl = slice(nt * 512 + hh * 256, nt * 512 + (hh + 1) * 256)
                        for j in range(EC):
                            nc.tensor.matmul(
                                ps1[:, cs], wst1[:, j, :], x_T[:, j, cs],
                                start=(j == 0), stop=(j == EC - 1),
                            )
                        for j in range(EC):
                            nc.tensor.matmul(
                                ps2[:, cs], wst2[:, j, :], x_T[:, j, cs],
                                start=(j == 0), stop=(j == EC - 1),
                            )
                        nc.vector.tensor_scalar_add(
                            qv[0:64, gsl], ps1[0:64, cs], b_q[:]
                        )
                        nc.vector.tensor_scalar_add(
                            kT[:, gsl], ps2[0:64, cs], b_k[:]
                        )
                        nc.vector.tensor_scalar_add(
                            qv[64:128, gsl], ps2[64:128, cs], b_v128[64:128, :]
                        )

                # v natural (+ones col), batched after the qkv stream
                for c4 in range(SC // 4):
                    psv = ps_t.tile([128, 512], F32R, name="pst")
                    for j in range(4):
                        c = c4 * 4 + j
                        nc.tensor.transpose(
                            psv[:, j * 64 : (j + 1) * 64],
                            qv[64:128, c * 128 : (c + 1) * 128],
                            ident_r[64:128, 64:128],
                        )
                    nc.scalar.copy(
                        v1[:, c4 * 4 : (c4 + 1) * 4, 0:DH],
                        psv[:, 0:256].rearrange("p (j f) -> p j f", j=4),
                    )

            # ---- phase 2: attention --------------------------------------
            with mask_ctx:
                with (
                    tc.tile_pool(name="u", bufs=6) as p_u,
                    tc.tile_pool(name="nz", bufs=3) as p_nz,
                    tc.tile_pool(name="osb", bufs=3) as p_o,
                    tc.tile_pool(name="ps_s", bufs=2, space="PSUM") as ps_s,
                    tc.tile_pool(name="ps_tm", bufs=2, space="PSUM") as ps_tm,
                    tc.tile_pool(name="ps_o", bufs=1, space="PSUM") as ps_o,
                    tc.tile_pool(name="ps_t2", bufs=1, space="PSUM") as ps_t2,
                ):
                    for bi, (q0, width) in enumerate(BLOCKS):
                        nmc = width // 128
                        mviews = [
                            m_tiles[q0 // 128 + mc][:].bitcast(BF16)
                            for mc in range(nmc)
                        ]
                        po_f = ps_o.tile([DH + 1, 512], F32, name="po")
                        po = po_f[:, :width]
                        # software-pipelined: transposes+scores of pair p
                        # are emitted before exp/cp/av of pair p-1
                        stage = []
                        for kp in range(KC // 2):
                            pstm_f = ps_tm.tile([128, 2, 512], BF16, name="pstm")
                            upair_f = p_u.tile([128, 2, 512], BF16, tag="u")
                            pss_f = ps_s.tile([128, 2, 512], F32, name="pss")
                            for j in range(2):
                                kc = kp * 2 + j
                                pstm = pstm_f[:, j, :width]
                                for mc in range(nmc):
                                    nc.tensor.matmul(
                                        pstm[:, mc * 128 : (mc + 1) * 128],
                                        mviews[mc][:, 2 * kc * 128 : 2 * (kc + 1) * 128 : 2],
                                        ident_bf[:],
                                        is_transpose=True,
                                    )
                                nc.tensor.matmul(
                                    pss_f[:, j, :width],
                                    kT[:, kc * 128 : (kc + 1) * 128],
                                    qv[0:64, q0 : q0 + width],
                                    start=True,
                                    stop=True,
                                )
                            stage.append((kp, pstm_f, upair_f, pss_f))
                            if len(stage) == 2 or kp == KC // 2 - 1:
                                for kp2, pstm2, upair2, pss2 in (
                                    list(stage) if kp == KC // 2 - 1 else stage[:1]
                                ):
                                    nc.scalar.activation(
                                        upair2[:, :, :width],
                                        pss2[:, :, :width], AF.Exp,
                                    )
                                    nc.vector.copy_predicated(
                                        upair2[:, :, :width],
                                        pstm2[:, :, :width].bitcast(I16),
                                        ones_bf[:, :, :width],
                                    )
                                    for j in range(2):
                                        kc = kp2 * 2 + j
                                        nc.tensor.matmul(
                                            po[:],
                                            v1[:, kc, :],
                                            upair2[:, j, :width],
                                            start=(kc == 0),
                                            stop=(kc == KC - 1),
                                        )
                                    stage.remove((kp2, pstm2, upair2, pss2))

                        numz_f = p_nz.tile([DH + 1, 512], F32, tag="nz")
                        numz = numz_f[:, :width]
                        nc.scalar.copy(numz[:], po[:])
                        o_sbn_f = p_o.tile([128, 4, DH], F32, tag="osb")
                        o_sbn = o_sbn_f[:, :nmc, :]
                        pt2 = ps_t2.tile([128, 4, DH + 1], F32, name="pt2")
                        for i in range(nmc):
                            nc.tensor.transpose(
                                pt2[:, i, :],
                                numz[:, i * 128 : (i + 1) * 128],
                                ident[0 : DH + 1, 0 : DH + 1],
                            )
                        r_cols = p_o.tile([128, 4], F32, tag="rcol")
                        nc.vector.reciprocal(
                            r_cols[:, 0:nmc], pt2[:, 0:nmc, DH]
                        )
                        for i in range(nmc):
                            nc.scalar.activation(
                                o_sbn[:, i, :], pt2[:, i, 0:DH], AF.Copy,
                                scale=r_cols[:, i : i + 1],
                            )
                        nc.gpsimd.dma_start(
                            o_out[q0 : q0 + width, :].rearrange(
                                "(i p) d -> p i d", p=128
                            ),
                            o_sbn[:],
                        )

    nc.finalize()
    return nc


_CACHED_NC = None


def _get_nc():
    global _CACHED_NC
    if _CACHED_NC is None:
        _CACHED_NC = build(bacc.Bacc())
    return _CACHED_NC


def kernel(x, mask, W, b, _trace=False, _tmpdir=None):
    """Full inputs in, full output out. Shards batch across 8 neuron cores."""
    x = np.ascontiguousarray(x, dtype=np.float32)
    mask = np.ascontiguousarray(mask, dtype=np.int32)
    W = np.ascontiguousarray(W, dtype=np.float32)
    b = np.ascontiguousarray(b, dtype=np.float32)
    assert x.shape == (B, S, E) and mask.shape == (B, S, S)

    nc = _get_nc()
    in_maps = [
        {"x": x[c], "mask": mask[c], "W": W, "b": b} for c in range(N_CORES)
    ]
    res = run_bass_kernel_spmd(
        nc, in_maps, list(range(N_CORES)), trace=_trace, tmpdir=_tmpdir
    )
    out = np.stack([res.results[c]["out"] for c in range(N_CORES)])
    if _trace:
        return out, res
    return out


# revision 30
# speedup vs baseline: 1.0685x; 1.0437x over previous
"""Trainium2 Bass kernel for nn_AttentionHead (B=8, S=2048, E=1024, Dh=64).

Sharding: data-parallel over batch B across the 8 NeuronCores (one batch
element per core); W/b replicated; results gathered on host.

Per-core computation in transposed orientation (scores^T[k,q]):
  qkv = x @ W + b  (1/sqrt(Dh) folded into W_q,b_q)
  scores^T = kT.T @ qT                  (PE, f32r, into a 2-bank PSUM pair)
  u = exp(scores^T)                     (ACT, one instr per k-chunk PAIR,
                                         PSUM -> SBUF bf16)
  u = where(mask^T, 1, u)               (DVE copy_predicated per pair;
                                         mask^T lives only in PSUM via
                                         bf16-bitcast PE transposes, re-viewed
                                         as int16 for the predicate)
  [num^T; Z] = [v | 1]^T @ u            (PE accumulate over k-chunks)
  out = (num * (1/Z))^T                 (ACT numz copy, PE transpose, DVE
                                         recip, ACT per-partition scale)

DMA: three parallel queues (SP HWDGE, ACT HWDGE, Pool SWDGE). In the cost
model each queue's transfers serialize at ~332 B/ns but different queues
overlap, so the 24.5 MiB of input streams in ~31 us instead of ~78 us.
  SP:   x half-blocks 0a..3a (4 x 1 MiB), then mask tiles 0,3,6,9,12,15
  ACT:  x half-blocks 0b..3b,             then mask tiles 2,5,8,11,14
  Pool: W, biases,                        then mask tiles 1,4,7,10,13,
        then per-block output stores
x transposes run at f32r (1.5 PE cycles/row instead of 2.0 for f32).
The span is then compute-bound: PE ~67 us busy (x-transpose 10.2, qkv 13.7,
scores 13.7, mask-transpose 13.7, av 13.7), DVE ~60, ACT ~57.
"""

import os
import sys

sys.path.insert(0, "/opt/trn_rl_repo")

import numpy as np

import concourse.bass as bass
import concourse.tile as tile
from concourse import bacc, mybir
from concourse.masks import make_identity
from concourse.bass_utils import run_bass_kernel_spmd

F32 = mybir.dt.float32
F32R = mybir.dt.float32r
BF16 = mybir.dt.bfloat16
F8 = mybir.dt.float8e4
I32 = mybir.dt.int32
I16 = mybir.dt.int16

# fp8 qkv: W is scaled by 16 into e4m3 normal range (sigma_W ~ 0.03 is
# subnormal territory otherwise); the 1/16 is folded into the psum->sbuf
# copies' scale. Precision is fine: q/k/v errors average down over the
# 1024-wide contraction (~0.3% rel), unlike fp8 on u which would not.
W8_SCALE = 16.0

B, S, E, DH = 8, 2048, 1024, 64
N_CORES = 8
SCALE = 1.0 / 8.0  # 1/sqrt(DH)

MM_DT = F32R

EC = E // 128     # 8   e-chunks
SC = S // 128     # 16  s-chunks (k-chunks)
QB = S // 512     # 4   x s-blocks
KC = SC

BLOCKS = [(0, 384), (384, 384), (768, 384), (1152, 384), (1536, 256), (1792, 256)]

AF = mybir.ActivationFunctionType
OP = mybir.AluOpType

# DMA transfer time occupies the issuing engine in the cost model, so bulk
# streams go on SP (no compute) and Pool (nearly idle); ACT carries only the
# small early W/bias loads its own wst/bias consumers need.
MASK_ENG = ["sync", "gpsimd"]


def build(nc: bass.Bass):
    x_in = nc.dram_tensor("x", [S, E], F32, kind="ExternalInput")
    m_in = nc.dram_tensor("mask", [S, S], I32, kind="ExternalInput")
    w_in = nc.dram_tensor("W", [E, 3 * DH], F32, kind="ExternalInput")
    b_in = nc.dram_tensor("b", [3 * DH], F32, kind="ExternalInput")
    o_out = nc.dram_tensor("out", [S, DH], F32, kind="ExternalOutput")

    trace_sim = bool(os.environ.get("TRN_TRACE_SIM"))
    with tile.TileContext(nc, trace_sim=trace_sim) as tc:
        with (
            tc.tile_pool(name="persist", bufs=1) as persist,
            tc.tile_pool(name="small", bufs=1) as small,
        ):
            # ---- constants / weights (Pool queue) ------------------------
            ident = persist.tile([128, 128], F32)
            make_identity(nc, ident)
            ident_bf = persist.tile([128, 128], BF16)
            nc.gpsimd.tensor_copy(ident_bf[:], ident[:])
            ident_r = persist.tile([128, 128], F32R)
            nc.gpsimd.tensor_copy(ident_r[:], ident[:])
            ones_bf = persist.tile([128, 2, 512], BF16)
            nc.gpsimd.memset(ones_bf[:], 1.0)

            w_raw = small.tile([128, EC, 3 * DH], F32)
            nc.scalar.dma_start(w_raw[:], w_in.rearrange("(o p) d -> p o d", p=128))
            # stacked f32r stationaries: wst1 = [W_q*scale | W_k],
            # wst2 = [W_k | W_v]  (fp8 would be ~5-9% error: per-element
            # quantization noise survives the contraction at full relative
            # strength)
            wst1 = persist.tile([128, EC, 128], F32R)
            wst2 = persist.tile([128, EC, 128], F32R)
            nc.vector.tensor_scalar_mul(wst1[:, :, 0:64], w_raw[:, :, 0:DH], SCALE)
            nc.scalar.copy(wst1[:, :, 64:128], w_raw[:, :, DH : 2 * DH])
            nc.vector.tensor_copy(wst2[:, :, 0:64], w_raw[:, :, DH : 2 * DH])
            nc.scalar.copy(wst2[:, :, 64:128], w_raw[:, :, 2 * DH : 3 * DH])

            # b_k is dropped: a k-bias adds a per-query constant to scores,
            # which softmax over k is invariant to (the bq.k0 cross-term is
            # kept via b_q). kT is then a pure psum copy.
            b_q_raw = small.tile([64, 1], F32)
            nc.scalar.dma_start(b_q_raw[:], b_in[0:64].unsqueeze(-1))
            b_q = persist.tile([64, 1], F32)
            nc.vector.tensor_scalar_mul(b_q[:], b_q_raw[:], SCALE)
            b_v128 = persist.tile([128, 1], F32)  # v bias on lanes 64-127
            nc.scalar.dma_start(b_v128[64:128, :], b_in[128:192].unsqueeze(-1))

            # warm the ACT exp table early
            warm = small.tile([1, 1], F32)
            nc.vector.memset(warm[:], 0.0)
            warm_o = small.tile([1, 1], BF16)
            nc.scalar.activation(warm_o[:], warm[:], AF.Exp)

            # persistent activations: qv rows 0-63 = q^T, rows 64-127 = v^T
            qv = persist.tile([128, S], MM_DT)
            kT = persist.tile([64, S], MM_DT)
            v1 = persist.tile([128, SC, DH + 1], BF16)  # v natural + ones col
            nc.gpsimd.memset(v1[:, :, DH : DH + 1], 1.0)

            from contextlib import ExitStack

            mask_ctx = ExitStack()
            # 9 rotating slots: tile n+9 reuses tile n's space, whose consumer
            # block finishes well before tile n+9 is needed (no deadlock: the
            # attend consumes tiles strictly in order)
            p_m = mask_ctx.enter_context(tc.tile_pool(name="mstage", bufs=9))

            # ---- phase 1: x -> x^T -> q/k/v ------------------------------
            with (
                tc.tile_pool(name="xnat", bufs=2) as p_xnat,
                tc.tile_pool(name="xT", bufs=2) as p_xT,
                tc.tile_pool(name="ps_t", bufs=4, space="PSUM") as ps_t,
                tc.tile_pool(name="ps_mm", bufs=4, space="PSUM") as ps_mm,
            ):
                x_nats = []
                for nt in range(QB):
                    # x half-blocks: "a" half on SP, "b" half on Pool; the
                    # first block is quartered so transposes start early
                    x_nat4 = p_xnat.tile([128, 4, E], F32R)
                    if nt == 0:
                        for q, eng in enumerate((nc.sync, nc.gpsimd) * 2):
                            eng.dma_start(
                                x_nat4[:, q : q + 1, :],
                                x_in[q * 128 : (q + 1) * 128, :]
                                .rearrange("(c p) e -> p c e", p=128)
                                .bitcast(F32R),
                            )
                    else:
                        for h, eng in ((0, nc.sync), (1, nc.gpsimd)):
                            eng.dma_start(
                                x_nat4[:, h * 2 : (h + 1) * 2, :],
                                x_in[nt * 512 + h * 256 : nt * 512 + (h + 1) * 256, :]
                                .rearrange("(c p) e -> p c e", p=128)
                                .bitcast(F32R),
                            )
                    x_nats.append(x_nat4)

                # mask tile DMAs, round-robin over the three queues, emitted
                # after x so each queue drains x first (program order)
                m_tiles = {}
                for mc in range(SC):
                    m_i32 = p_m.tile([128, S], I32)
                    eng = getattr(nc, MASK_ENG[mc % 2])
                    eng.dma_start(m_i32[:], m_in[mc * 128 : (mc + 1) * 128, :])
                    m_tiles[mc] = m_i32

                for nt in range(QB):
                    x_nat4 = x_nats[nt]
                    x_T = p_xT.tile([128, EC, 512], F32R)
                    # all transposes+copies of the block first, qkv after:
                    # keeps the in-order PE queue free of qkv instructions
                    # that would stall waiting on the copies
                    for c4 in range(4):
                        for g in range(2):
                            pst = ps_t.tile([128, 512], F32R, name="pst")
                            for j4 in range(4):
                                j = g * 4 + j4
                                nc.tensor.transpose(
                                    pst[:, j4 * 128 : (j4 + 1) * 128],
                                    x_nat4[:, c4, j * 128 : (j + 1) * 128],
                                    ident_r[:],
                                )
                            dst = x_T[
                                :, g * 4 : (g + 1) * 4,
                                c4 * 128 : (c4 + 1) * 128,
                            ]
                            src = pst[:].rearrange("p (j f) -> p j f", j=4)
                            if (c4 + g) % 2 == 0:
                                nc.vector.tensor_copy(dst, src)
                            else:
                                nc.scalar.copy(dst, src)  # ACT

                    for p2 in range(2):
                        ps1 = ps_mm.tile([128, 256], F32, name="psmm")
                        ps2 = ps_mm.tile([128, 256], F32, name="psmm")
                        cs = slice(p2 * 256, (p2 + 1) * 256)
                        gsl = slice(nt * 512 + p2 * 256, nt * 512 + (p2 + 1) * 256)
                        for j in range(EC):
                            nc.tensor.matmul(
                                ps1[:], wst1[:, j, :], x_T[:, j, cs],
                                start=(j == 0), stop=(j == EC - 1),
                            )
                        for j in range(EC):
                            nc.tensor.matmul(
                                ps2[:], wst2[:, j, :], x_T[:, j, cs],
                                start=(j == 0), stop=(j == EC - 1),
                            )
                        # kT is a pure copy (k-bias dropped: softmax-invariant)
                        nc.vector.tensor_scalar_add(
                            qv[0:64, gsl], ps1[0:64, :], b_q[:]
                        )
                        nc.scalar.copy(kT[:, gsl], ps2[0:64, :])
                        nc.vector.tensor_scalar_add(
                            qv[64:128, gsl], ps2[64:128, :], b_v128[64:128, :]
                        )

                    # v natural (+ones col) for this block's four s-chunks
                    psv = ps_mm.tile([128, 256], F32R, name="psmm")
                    for j in range(4):
                        c = nt * 4 + j
                        nc.tensor.transpose(
                            psv[:, j * 64 : (j + 1) * 64],
                            qv[64:128, c * 128 : (c + 1) * 128],
                            ident_r[64:128, 64:128],
                        )
                    nc.scalar.copy(
                        v1[:, nt * 4 : (nt + 1) * 4, 0:DH],
                        psv[:, 0:256].rearrange("p (j f) -> p j f", j=4),
                    )

            # ---- phase 2: attention --------------------------------------
            with mask_ctx:
                with (
                    tc.tile_pool(name="u", bufs=6) as p_u,
                    tc.tile_pool(name="nz", bufs=3) as p_nz,
                    tc.tile_pool(name="osb", bufs=3) as p_o,
                    tc.tile_pool(name="ps_s", bufs=2, space="PSUM") as ps_s,
                    tc.tile_pool(name="ps_tm", bufs=2, space="PSUM") as ps_tm,
                    tc.tile_pool(name="ps_o", bufs=1, space="PSUM") as ps_o,
                    tc.tile_pool(name="ps_t2", bufs=1, space="PSUM") as ps_t2,
                ):
                    for bi, (q0, width) in enumerate(BLOCKS):
                        nmc = width // 128
                        mviews = [
                            m_tiles[q0 // 128 + mc][:].bitcast(BF16)
                            for mc in range(nmc)
                        ]
                        po_f = ps_o.tile([DH + 1, 512], F32, name="po")
                        po = po_f[:, :width]
                        # software-pipelined: transposes+scores of pair p
                        # are emitted before exp/cp/av of pair p-1
                        stage = []
                        for kp in range(KC // 2):
                            pstm_f = ps_tm.tile([128, 2, 512], BF16, name="pstm")
                            upair_f = p_u.tile([128, 2, 512], BF16, tag="u")
                            pss_f = ps_s.tile([128, 2, 512], F32, name="pss")
                            for j in range(2):
                                kc = kp * 2 + j
                                pstm = pstm_f[:, j, :width]
                                for mc in range(nmc):
                                    nc.tensor.matmul(
                                        pstm[:, mc * 128 : (mc + 1) * 128],
                                        mviews[mc][:, 2 * kc * 128 : 2 * (kc + 1) * 128 : 2],
                                        ident_bf[:],
                                        is_transpose=True,
                                    )
                                nc.tensor.matmul(
                                    pss_f[:, j, :width],
                                    kT[:, kc * 128 : (kc + 1) * 128],
                                    qv[0:64, q0 : q0 + width],
                                    start=True,
                                    stop=True,
                                )
                            stage.append((kp, pstm_f, upair_f, pss_f))
                            if len(stage) == 2 or kp == KC // 2 - 1:
                                for kp2, pstm2, upair2, pss2 in (
                                    list(stage) if kp == KC // 2 - 1 else stage[:1]
                                ):
                                    nc.scalar.activation(
                                        upair2[:, :, :width],
                                        pss2[:, :, :width], AF.Exp,
                                    )
                                    nc.vector.copy_predicated(
                                        upair2[:, :, :width],
                                        pstm2[:, :, :width].bitcast(I16),
                                        ones_bf[:, :, :width],
                                    )
                                    for j in range(2):
                                        kc = kp2 * 2 + j
                                        nc.tensor.matmul(
                                            po[:],
                                            v1[:, kc, :],
                                            upair2[:, j, :width],
                                            start=(kc == 0),
                                            stop=(kc == KC - 1),
                                        )
                                    stage.remove((kp2, pstm2, upair2, pss2))

                        numz_f = p_nz.tile([DH + 1, 512], F32, tag="nz")
                        numz = numz_f[:, :width]
                        if bi % 2 == 0:
                            nc.vector.tensor_copy(numz[:], po[:])
                        else:
                            nc.scalar.copy(numz[:], po[:])
                        o_sbn_f = p_o.tile([128, 4, DH], F32, tag="osb")
                        o_sbn = o_sbn_f[:, :nmc, :]
                        pt2 = ps_t2.tile([128, 4, DH + 1], F32, name="pt2")
                        for i in range(nmc):
                            nc.tensor.transpose(
                                pt2[:, i, :],
                                numz[:, i * 128 : (i + 1) * 128],
                                ident[0 : DH + 1, 0 : DH + 1],
                            )
                        r_cols = p_o.tile([128, 4], F32, tag="rcol")
                        nc.vector.reciprocal(
                            r_cols[:, 0:nmc], pt2[:, 0:nmc, DH]
                        )
                        for i in range(nmc):
                            if bi % 2 == 0:
                                nc.vector.tensor_scalar_mul(
                                    o_sbn[:, i, :], pt2[:, i, 0:DH],
                                    r_cols[:, i : i + 1],
                                )
                            else:
                                nc.scalar.activation(
                                    o_sbn[:, i, :], pt2[:, i, 0:DH], AF.Copy,
                                    scale=r_cols[:, i : i + 1],
                                )
                        nc.gpsimd.dma_start(
                            o_out[q0 : q0 + width, :].rearrange(
                                "(i p) d -> p i d", p=128
                            ),
                            o_sbn[:],
                        )

    nc.finalize()
    return nc


_CACHED_NC = None


def _get_nc():
    global _CACHED_NC
    if _CACHED_NC is None:
        _CACHED_NC = build(bacc.Bacc())
    return _CACHED_NC


def kernel(x, mask, W, b, _trace=False, _tmpdir=None):
    """Full inputs in, full output out. Shards batch across 8 neuron cores."""
    x = np.ascontiguousarray(x, dtype=np.float32)
    mask = np.ascontiguousarray(mask, dtype=np.int32)
    W = np.ascontiguousarray(W, dtype=np.float32)
    b = np.ascontiguousarray(b, dtype=np.float32)
    assert x.shape == (B, S, E) and mask.shape == (B, S, S)

    nc = _get_nc()
    in_maps = [
        {"x": x[c], "mask": mask[c], "W": W, "b": b} for c in range(N_CORES)
    ]
    res = run_bass_kernel_spmd(
        nc, in_maps, list(range(N_CORES)), trace=_trace, tmpdir=_tmpdir
    )
    out = np.stack([res.results[c]["out"] for c in range(N_CORES)])
    if _trace:
        return out, res
    return out


# revision 39
# speedup vs baseline: 1.0752x; 1.0063x over previous
"""Trainium2 Bass kernel for nn_AttentionHead (B=8, S=2048, E=1024, Dh=64).

Sharding: data-parallel over batch B across the 8 NeuronCores (one batch
element per core); W/b replicated; results gathered on host.

Per-core computation in transposed orientation (scores^T[k,q]):
  qkv = x @ W + b  (1/sqrt(Dh) folded into W_q,b_q)
  scores^T = kT.T @ qT                  (PE, f32r, into a 2-bank PSUM pair)
  u = exp(scores^T)                     (ACT, one instr per k-chunk PAIR,
                                         PSUM -> SBUF bf16)
  u = where(mask^T, 1, u)               (DVE copy_predicated per pair;
                                         mask^T lives only in PSUM via
                                         bf16-bitcast PE transposes, re-viewed
                                         as int16 for the predicate)
  [num^T; Z] = [v | 1]^T @ u            (PE accumulate over k-chunks)
  out = (num * (1/Z))^T                 (ACT numz copy, PE transpose, DVE
                                         recip, ACT per-partition scale)

DMA: three parallel queues (SP HWDGE, ACT HWDGE, Pool SWDGE). In the cost
model each queue's transfers serialize at ~332 B/ns but different queues
overlap, so the 24.5 MiB of input streams in ~31 us instead of ~78 us.
  SP:   x half-blocks 0a..3a (4 x 1 MiB), then mask tiles 0,3,6,9,12,15
  ACT:  x half-blocks 0b..3b,             then mask tiles 2,5,8,11,14
  Pool: W, biases,                        then mask tiles 1,4,7,10,13,
        then per-block output stores
x transposes run at f32r (1.5 PE cycles/row instead of 2.0 for f32).
The span is then compute-bound: PE ~67 us busy (x-transpose 10.2, qkv 13.7,
scores 13.7, mask-transpose 13.7, av 13.7), DVE ~60, ACT ~57.
"""

import os
import sys

sys.path.insert(0, "/opt/trn_rl_repo")

import numpy as np

import concourse.bass as bass
import concourse.tile as tile
from concourse import bacc, mybir
from concourse.masks import make_identity
from concourse.bass_utils import run_bass_kernel_spmd

F32 = mybir.dt.float32
F32R = mybir.dt.float32r
BF16 = mybir.dt.bfloat16
F8 = mybir.dt.float8e4
I32 = mybir.dt.int32
I16 = mybir.dt.int16

# fp8 qkv: W is scaled by 16 into e4m3 normal range (sigma_W ~ 0.03 is
# subnormal territory otherwise); the 1/16 is folded into the psum->sbuf
# copies' scale. Precision is fine: q/k/v errors average down over the
# 1024-wide contraction (~0.3% rel), unlike fp8 on u which would not.
W8_SCALE = 16.0

B, S, E, DH = 8, 2048, 1024, 64
N_CORES = 8
SCALE = 1.0 / 8.0  # 1/sqrt(DH)

MM_DT = F32R

EC = E // 128     # 8   e-chunks
SC = S // 128     # 16  s-chunks (k-chunks)
QB = S // 512     # 4   x s-blocks
KC = SC

BLOCKS = [(0, 512), (512, 512), (1024, 512), (1536, 512)]

AF = mybir.ActivationFunctionType
OP = mybir.AluOpType

# DMA transfer time occupies the issuing engine in the cost model, so bulk
# streams go on SP (no compute) and Pool (nearly idle); ACT carries only the
# small early W/bias loads its own wst/bias consumers need.
MASK_ENG = ["sync", "gpsimd"]


def build(nc: bass.Bass):
    x_in = nc.dram_tensor("x", [S, E], F32, kind="ExternalInput")
    m_in = nc.dram_tensor("mask", [S, S], I32, kind="ExternalInput")
    w_in = nc.dram_tensor("W", [E, 3 * DH], F32, kind="ExternalInput")
    b_in = nc.dram_tensor("b", [3 * DH], F32, kind="ExternalInput")
    o_out = nc.dram_tensor("out", [S, DH], F32, kind="ExternalOutput")

    trace_sim = bool(os.environ.get("TRN_TRACE_SIM"))
    with tile.TileContext(nc, trace_sim=trace_sim) as tc:
        with (
            tc.tile_pool(name="persist", bufs=1) as persist,
            tc.tile_pool(name="small", bufs=1) as small,
        ):
            # ---- constants / weights (Pool queue) ------------------------
            ident = persist.tile([128, 128], F32)
            make_identity(nc, ident)
            ident_bf = persist.tile([128, 128], BF16)
            nc.vector.tensor_copy(ident_bf[:], ident[:])
            ident_r = persist.tile([128, 128], F32R)
            nc.vector.tensor_copy(ident_r[:], ident[:])
            ones_bf = persist.tile([128, 2, 512], BF16)
            nc.gpsimd.memset(ones_bf[:], 1.0)

            # PE clock warmup: ~3us of dummy matmuls from t~0.2 so the PE
            # p-state is fully ramped when the first real transposes arrive
            warm_w = small.tile([128, 128], BF16)
            nc.vector.memset(warm_w[:], 0.0)
            with tc.tile_pool(name="ps_w", bufs=1, space="PSUM") as ps_w:
                ps_warm = ps_w.tile([128, 128], F32, name="psw")
                for _ in range(12):
                    nc.tensor.matmul(
                        ps_warm[:], warm_w[:], warm_w[:], start=True, stop=True
                    )

            w_raw = small.tile([128, EC, 3 * DH], F32)
            nc.scalar.dma_start(w_raw[:], w_in.rearrange("(o p) d -> p o d", p=128))
            # stacked f32r stationaries: wst1 = [W_q*scale | W_k],
            # wst2 = [W_k | W_v]  (fp8 would be ~5-9% error: per-element
            # quantization noise survives the contraction at full relative
            # strength)
            wst1 = persist.tile([128, EC, 128], F32R)
            wst2 = persist.tile([128, EC, 128], F32R)
            nc.vector.tensor_scalar_mul(wst1[:, :, 0:64], w_raw[:, :, 0:DH], SCALE)
            nc.scalar.copy(wst1[:, :, 64:128], w_raw[:, :, DH : 2 * DH])
            nc.vector.tensor_copy(wst2[:, :, 0:64], w_raw[:, :, DH : 2 * DH])
            nc.scalar.copy(wst2[:, :, 64:128], w_raw[:, :, 2 * DH : 3 * DH])

            # b_k is dropped: a k-bias adds a per-query constant to scores,
            # which softmax over k is invariant to (the bq.k0 cross-term is
            # kept via b_q). kT is then a pure psum copy.
            b_q_raw = small.tile([64, 1], F32)
            nc.scalar.dma_start(b_q_raw[:], b_in[0:64].unsqueeze(-1))
            b_q = persist.tile([64, 1], F32)
            nc.vector.tensor_scalar_mul(b_q[:], b_q_raw[:], SCALE)
            b_v128 = persist.tile([128, 1], F32)  # v bias on lanes 64-127
            nc.scalar.dma_start(b_v128[64:128, :], b_in[128:192].unsqueeze(-1))

            # warm the ACT exp table early
            warm = small.tile([1, 1], F32)
            nc.vector.memset(warm[:], 0.0)
            warm_o = small.tile([1, 1], BF16)
            nc.scalar.activation(warm_o[:], warm[:], AF.Exp)

            # persistent activations: qv rows 0-63 = q^T, rows 64-127 = v^T
            qv = persist.tile([128, S], MM_DT)
            kT = persist.tile([64, S], MM_DT)
            v1 = persist.tile([128, SC, DH + 1], BF16)  # v natural + ones col
            nc.gpsimd.memset(v1[:, :, DH : DH + 1], 1.0)

            from contextlib import ExitStack

            mask_ctx = ExitStack()
            # 9 rotating slots: tile n+9 reuses tile n's space, whose consumer
            # block finishes well before tile n+9 is needed (no deadlock: the
            # attend consumes tiles strictly in order)
            p_m = mask_ctx.enter_context(tc.tile_pool(name="mstage", bufs=9))

            # ---- phase 1: x -> x^T -> q/k/v ------------------------------
            with (
                tc.tile_pool(name="xnat", bufs=2) as p_xnat,
                tc.tile_pool(name="xT", bufs=2) as p_xT,
                tc.tile_pool(name="ps_t", bufs=4, space="PSUM") as ps_t,
                tc.tile_pool(name="ps_mm", bufs=4, space="PSUM") as ps_mm,
            ):
                x_nats = []
                for nt in range(QB):
                    # x half-blocks: "a" half on SP, "b" half on Pool; the
                    # first block is quartered so transposes start early
                    x_nat4 = p_xnat.tile([128, 4, E], F32R)
                    if nt == 0:
                        for q, eng in enumerate((nc.sync, nc.gpsimd) * 2):
                            eng.dma_start(
                                x_nat4[:, q : q + 1, :],
                                x_in[q * 128 : (q + 1) * 128, :]
                                .rearrange("(c p) e -> p c e", p=128)
                                .bitcast(F32R),
                            )
                    else:
                        for h, eng in ((0, nc.sync), (1, nc.gpsimd)):
                            eng.dma_start(
                                x_nat4[:, h * 2 : (h + 1) * 2, :],
                                x_in[nt * 512 + h * 256 : nt * 512 + (h + 1) * 256, :]
                                .rearrange("(c p) e -> p c e", p=128)
                                .bitcast(F32R),
                            )
                    x_nats.append(x_nat4)

                # mask tile DMAs, round-robin over the three queues, emitted
                # after x so each queue drains x first (program order)
                m_tiles = {}
                for mc in range(SC):
                    m_i32 = p_m.tile([128, S], I32)
                    eng = getattr(nc, MASK_ENG[mc % 2])
                    eng.dma_start(m_i32[:], m_in[mc * 128 : (mc + 1) * 128, :])
                    m_tiles[mc] = m_i32

                for nt in range(QB):
                    x_nat4 = x_nats[nt]
                    x_T = p_xT.tile([128, EC, 512], F32R)
                    # all transposes+copies of the block first, qkv after:
                    # keeps the in-order PE queue free of qkv instructions
                    # that would stall waiting on the copies
                    for c4 in range(4):
                        for g in range(2):
                            pst = ps_t.tile([128, 512], F32R, name="pst")
                            for j4 in range(4):
                                j = g * 4 + j4
                                nc.tensor.transpose(
                                    pst[:, j4 * 128 : (j4 + 1) * 128],
                                    x_nat4[:, c4, j * 128 : (j + 1) * 128],
                                    ident_r[:],
                                )
                            dst = x_T[
                                :, g * 4 : (g + 1) * 4,
                                c4 * 128 : (c4 + 1) * 128,
                            ]
                            src = pst[:].rearrange("p (j f) -> p j f", j=4)
                            if (c4 + g) % 2 == 0:
                                nc.vector.tensor_copy(dst, src)
                            else:
                                nc.scalar.copy(dst, src)  # ACT

                    for p2 in range(2):
                        ps1 = ps_mm.tile([128, 256], F32, name="psmm")
                        ps2 = ps_mm.tile([128, 256], F32, name="psmm")
                        cs = slice(p2 * 256, (p2 + 1) * 256)
                        gsl = slice(nt * 512 + p2 * 256, nt * 512 + (p2 + 1) * 256)
                        for j in range(EC):
                            nc.tensor.matmul(
                                ps1[:], wst1[:, j, :], x_T[:, j, cs],
                                start=(j == 0), stop=(j == EC - 1),
                            )
                        for j in range(EC):
                            nc.tensor.matmul(
                                ps2[:], wst2[:, j, :], x_T[:, j, cs],
                                start=(j == 0), stop=(j == EC - 1),
                            )
                        # kT is a pure copy (k-bias dropped: softmax-invariant)
                        nc.vector.tensor_scalar_add(
                            qv[0:64, gsl], ps1[0:64, :], b_q[:]
                        )
                        nc.scalar.copy(kT[:, gsl], ps2[0:64, :])
                        nc.vector.tensor_scalar_add(
                            qv[64:128, gsl], ps2[64:128, :], b_v128[64:128, :]
                        )

                    # v natural (+ones col) for this block's four s-chunks
                    psv = ps_mm.tile([128, 256], F32R, name="psmm")
                    for j in range(4):
                        c = nt * 4 + j
                        nc.tensor.transpose(
                            psv[:, j * 64 : (j + 1) * 64],
                            qv[64:128, c * 128 : (c + 1) * 128],
                            ident_r[64:128, 64:128],
                        )
                    nc.scalar.copy(
                        v1[:, nt * 4 : (nt + 1) * 4, 0:DH],
                        psv[:, 0:256].rearrange("p (j f) -> p j f", j=4),
                    )

            # ---- phase 2: attention --------------------------------------
            with mask_ctx:
                with (
                    tc.tile_pool(name="u", bufs=6) as p_u,
                    tc.tile_pool(name="nz", bufs=3) as p_nz,
                    tc.tile_pool(name="osb", bufs=3) as p_o,
                    tc.tile_pool(name="ps_s", bufs=2, space="PSUM") as ps_s,
                    tc.tile_pool(name="ps_tm", bufs=2, space="PSUM") as ps_tm,
                    tc.tile_pool(name="ps_o", bufs=1, space="PSUM") as ps_o,
                    tc.tile_pool(name="ps_t2", bufs=1, space="PSUM") as ps_t2,
                ):
                    for bi, (q0, width) in enumerate(BLOCKS):
                        nmc = width // 128
                        mviews = [
                            m_tiles[q0 // 128 + mc][:].bitcast(BF16)
                            for mc in range(nmc)
                        ]
                        po_f = ps_o.tile([DH + 1, 512], F32, name="po")
                        po = po_f[:, :width]
                        # software-pipelined: transposes+scores of pair p
                        # are emitted before exp/cp/av of pair p-1
                        stage = []
                        for kp in range(KC // 2):
                            pstm_f = ps_tm.tile([128, 2, 512], BF16, name="pstm")
                            upair_f = p_u.tile([128, 2, 512], BF16, tag="u")
                            pss_f = ps_s.tile([128, 2, 512], F32, name="pss")
                            for j in range(2):
                                kc = kp * 2 + j
                                pstm = pstm_f[:, j, :width]
                                for mc in range(nmc):
                                    nc.tensor.matmul(
                                        pstm[:, mc * 128 : (mc + 1) * 128],
                                        mviews[mc][:, 2 * kc * 128 : 2 * (kc + 1) * 128 : 2],
                                        ident_bf[:],
                                        is_transpose=True,
                                    )
                                nc.tensor.matmul(
                                    pss_f[:, j, :width],
                                    kT[:, kc * 128 : (kc + 1) * 128],
                                    qv[0:64, q0 : q0 + width],
                                    start=True,
                                    stop=True,
                                )
                            stage.append((kp, pstm_f, upair_f, pss_f))
                            if len(stage) == 2 or kp == KC // 2 - 1:
                                for kp2, pstm2, upair2, pss2 in (
                                    list(stage) if kp == KC // 2 - 1 else stage[:1]
                                ):
                                    nc.scalar.activation(
                                        upair2[:, :, :width],
                                        pss2[:, :, :width], AF.Exp,
                                    )
                                    nc.vector.copy_predicated(
                                        upair2[:, :, :width],
                                        pstm2[:, :, :width].bitcast(I16),
                                        ones_bf[:, :, :width],
                                    )
                                    for j in range(2):
                                        kc = kp2 * 2 + j
                                        nc.tensor.matmul(
                                            po[:],
                                            v1[:, kc, :],
                                            upair2[:, j, :width],
                                            start=(kc == 0),
                                            stop=(kc == KC - 1),
                                        )
                                    stage.remove((kp2, pstm2, upair2, pss2))

                        numz_f = p_nz.tile([DH + 1, 512], F32, tag="nz")
                        numz = numz_f[:, :width]
                        if bi % 2 == 0:
                            nc.vector.tensor_copy(numz[:], po[:])
                        else:
                            nc.scalar.copy(numz[:], po[:])
                        o_sbn_f = p_o.tile([128, 4, DH], F32, tag="osb")
                        o_sbn = o_sbn_f[:, :nmc, :]
                        pt2 = ps_t2.tile([128, 4, DH + 1], F32, name="pt2")
                        for i in range(nmc):
                            nc.tensor.transpose(
                                pt2[:, i, :],
                                numz[:, i * 128 : (i + 1) * 128],
                                ident[0 : DH + 1, 0 : DH + 1],
                            )
                        r_cols = p_o.tile([128, 4], F32, tag="rcol")
                        nc.vector.reciprocal(
                            r_cols[:, 0:nmc], pt2[:, 0:nmc, DH]
                        )
                        for i in range(nmc):
                            if bi % 2 == 0:
                                nc.vector.tensor_scalar_mul(
                                    o_sbn[:, i, :], pt2[:, i, 0:DH],
                                    r_cols[:, i : i + 1],
                                )
                            else:
                                nc.scalar.activation(
                                    o_sbn[:, i, :], pt2[:, i, 0:DH], AF.Copy,
                                    scale=r_cols[:, i : i + 1],
                                )
                        out_eng = nc.sync if bi == len(BLOCKS) - 1 else nc.gpsimd
                        out_eng.dma_start(
                            o_out[q0 : q0 + width, :].rearrange(
                                "(i p) d -> p i d", p=128
                            ),
                            o_sbn[:],
                        )

    nc.finalize()
    return nc


_CACHED_NC = None


def _get_nc():
    global _CACHED_NC
    if _CACHED_NC is None:
        _CACHED_NC = build(bacc.Bacc())
    return _CACHED_NC


def kernel(x, mask, W, b, _trace=False, _tmpdir=None):
    """Full inputs in, full output out. Shards batch across 8 neuron cores."""
    x = np.ascontiguousarray(x, dtype=np.float32)
    mask = np.ascontiguousarray(mask, dtype=np.int32)
    W = np.ascontiguousarray(W, dtype=np.float32)
    b = np.ascontiguousarray(b, dtype=np.float32)
    assert x.shape == (B, S, E) and mask.shape == (B, S, S)

    nc = _get_nc()
    in_maps = [
        {"x": x[c], "mask": mask[c], "W": W, "b": b} for c in range(N_CORES)
    ]
    res = run_bass_kernel_spmd(
        nc, in_maps, list(range(N_CORES)), trace=_trace, tmpdir=_tmpdir
    )
    out = np.stack([res.results[c]["out"] for c in range(N_CORES)])
    if _trace:
        return out, res
    return out


# revision 40
# speedup vs baseline: 1.0817x; 1.0061x over previous
"""Trainium2 Bass kernel for nn_AttentionHead (B=8, S=2048, E=1024, Dh=64).

Sharding: data-parallel over batch B across the 8 NeuronCores (one batch
element per core); W/b replicated; results gathered on host.

Per-core computation in transposed orientation (scores^T[k,q]):
  qkv = x @ W + b  (1/sqrt(Dh) folded into W_q,b_q)
  scores^T = kT.T @ qT                  (PE, f32r, into a 2-bank PSUM pair)
  u = exp(scores^T)                     (ACT, one instr per k-chunk PAIR,
                                         PSUM -> SBUF bf16)
  u = where(mask^T, 1, u)               (DVE copy_predicated per pair;
                                         mask^T lives only in PSUM via
                                         bf16-bitcast PE transposes, re-viewed
                                         as int16 for the predicate)
  [num^T; Z] = [v | 1]^T @ u            (PE accumulate over k-chunks)
  out = (num * (1/Z))^T                 (ACT numz copy, PE transpose, DVE
                                         recip, ACT per-partition scale)

DMA: three parallel queues (SP HWDGE, ACT HWDGE, Pool SWDGE). In the cost
model each queue's transfers serialize at ~332 B/ns but different queues
overlap, so the 24.5 MiB of input streams in ~31 us instead of ~78 us.
  SP:   x half-blocks 0a..3a (4 x 1 MiB), then mask tiles 0,3,6,9,12,15
  ACT:  x half-blocks 0b..3b,             then mask tiles 2,5,8,11,14
  Pool: W, biases,                        then mask tiles 1,4,7,10,13,
        then per-block output stores
x transposes run at f32r (1.5 PE cycles/row instead of 2.0 for f32).
The span is then compute-bound: PE ~67 us busy (x-transpose 10.2, qkv 13.7,
scores 13.7, mask-transpose 13.7, av 13.7), DVE ~60, ACT ~57.
"""

import os
import sys

sys.path.insert(0, "/opt/trn_rl_repo")

import numpy as np

import concourse.bass as bass
import concourse.tile as tile
from concourse import bacc, mybir
from concourse.masks import make_identity
from concourse.bass_utils import run_bass_kernel_spmd

F32 = mybir.dt.float32
F32R = mybir.dt.float32r
BF16 = mybir.dt.bfloat16
F8 = mybir.dt.float8e4
I32 = mybir.dt.int32
I16 = mybir.dt.int16

# fp8 qkv: W is scaled by 16 into e4m3 normal range (sigma_W ~ 0.03 is
# subnormal territory otherwise); the 1/16 is folded into the psum->sbuf
# copies' scale. Precision is fine: q/k/v errors average down over the
# 1024-wide contraction (~0.3% rel), unlike fp8 on u which would not.
W8_SCALE = 16.0

B, S, E, DH = 8, 2048, 1024, 64
N_CORES = 8
SCALE = 1.0 / 8.0  # 1/sqrt(DH)

MM_DT = F32R

EC = E // 128     # 8   e-chunks
SC = S // 128     # 16  s-chunks (k-chunks)
QB = S // 512     # 4   x s-blocks
KC = SC

BLOCKS = [(0, 512), (512, 512), (1024, 512), (1536, 256), (1792, 256)]

AF = mybir.ActivationFunctionType
OP = mybir.AluOpType

# DMA transfer time occupies the issuing engine in the cost model, so bulk
# streams go on SP (no compute) and Pool (nearly idle); ACT carries only the
# small early W/bias loads its own wst/bias consumers need.
MASK_ENG = ["sync", "gpsimd"]


def build(nc: bass.Bass):
    x_in = nc.dram_tensor("x", [S, E], F32, kind="ExternalInput")
    m_in = nc.dram_tensor("mask", [S, S], I32, kind="ExternalInput")
    w_in = nc.dram_tensor("W", [E, 3 * DH], F32, kind="ExternalInput")
    b_in = nc.dram_tensor("b", [3 * DH], F32, kind="ExternalInput")
    o_out = nc.dram_tensor("out", [S, DH], F32, kind="ExternalOutput")

    trace_sim = bool(os.environ.get("TRN_TRACE_SIM"))
    with tile.TileContext(nc, trace_sim=trace_sim) as tc:
        with (
            tc.tile_pool(name="persist", bufs=1) as persist,
            tc.tile_pool(name="small", bufs=1) as small,
        ):
            # ---- constants / weights (Pool queue) ------------------------
            ident = persist.tile([128, 128], F32)
            make_identity(nc, ident)
            ident_bf = persist.tile([128, 128], BF16)
            nc.vector.tensor_copy(ident_bf[:], ident[:])
            ident_r = persist.tile([128, 128], F32R)
            nc.vector.tensor_copy(ident_r[:], ident[:])
            ones_bf = persist.tile([128, 2, 512], BF16)
            nc.gpsimd.memset(ones_bf[:], 1.0)

            # PE clock warmup: ~3us of dummy matmuls from t~0.2 so the PE
            # p-state is fully ramped when the first real transposes arrive
            warm_w = small.tile([128, 128], BF16)
            nc.vector.memset(warm_w[:], 0.0)
            with tc.tile_pool(name="ps_w", bufs=1, space="PSUM") as ps_w:
                ps_warm = ps_w.tile([128, 128], F32, name="psw")
                for _ in range(12):
                    nc.tensor.matmul(
                        ps_warm[:], warm_w[:], warm_w[:], start=True, stop=True
                    )

            w_raw = small.tile([128, EC, 3 * DH], F32)
            nc.scalar.dma_start(w_raw[:], w_in.rearrange("(o p) d -> p o d", p=128))
            # stacked f32r stationaries: wst1 = [W_q*scale | W_k],
            # wst2 = [W_k | W_v]  (fp8 would be ~5-9% error: per-element
            # quantization noise survives the contraction at full relative
            # strength)
            wst1 = persist.tile([128, EC, 128], F32R)
            wst2 = persist.tile([128, EC, 128], F32R)
            nc.vector.tensor_scalar_mul(wst1[:, :, 0:64], w_raw[:, :, 0:DH], SCALE)
            nc.scalar.copy(wst1[:, :, 64:128], w_raw[:, :, DH : 2 * DH])
            nc.vector.tensor_copy(wst2[:, :, 0:64], w_raw[:, :, DH : 2 * DH])
            nc.scalar.copy(wst2[:, :, 64:128], w_raw[:, :, 2 * DH : 3 * DH])

            # b_k is dropped: a k-bias adds a per-query constant to scores,
            # which softmax over k is invariant to (the bq.k0 cross-term is
            # kept via b_q). kT is then a pure psum copy.
            b_q_raw = small.tile([64, 1], F32)
            nc.scalar.dma_start(b_q_raw[:], b_in[0:64].unsqueeze(-1))
            b_q = persist.tile([64, 1], F32)
            nc.vector.tensor_scalar_mul(b_q[:], b_q_raw[:], SCALE)
            b_v128 = persist.tile([128, 1], F32)  # v bias on lanes 64-127
            nc.scalar.dma_start(b_v128[64:128, :], b_in[128:192].unsqueeze(-1))

            # warm the ACT exp table early
            warm = small.tile([1, 1], F32)
            nc.vector.memset(warm[:], 0.0)
            warm_o = small.tile([1, 1], BF16)
            nc.scalar.activation(warm_o[:], warm[:], AF.Exp)

            # persistent activations: qv rows 0-63 = q^T, rows 64-127 = v^T
            qv = persist.tile([128, S], MM_DT)
            kT = persist.tile([64, S], MM_DT)
            v1 = persist.tile([128, SC, DH + 1], BF16)  # v natural + ones col
            nc.gpsimd.memset(v1[:, :, DH : DH + 1], 1.0)

            from contextlib import ExitStack

            mask_ctx = ExitStack()
            # 9 rotating slots: tile n+9 reuses tile n's space, whose consumer
            # block finishes well before tile n+9 is needed (no deadlock: the
            # attend consumes tiles strictly in order)
            p_m = mask_ctx.enter_context(tc.tile_pool(name="mstage", bufs=9))

            # ---- phase 1: x -> x^T -> q/k/v ------------------------------
            with (
                tc.tile_pool(name="xnat", bufs=2) as p_xnat,
                tc.tile_pool(name="xT", bufs=2) as p_xT,
                tc.tile_pool(name="ps_t", bufs=4, space="PSUM") as ps_t,
                tc.tile_pool(name="ps_mm", bufs=4, space="PSUM") as ps_mm,
            ):
                x_nats = []
                for nt in range(QB):
                    # x half-blocks: "a" half on SP, "b" half on Pool; the
                    # first block is quartered so transposes start early
                    x_nat4 = p_xnat.tile([128, 4, E], F32R)
                    if nt == 0:
                        for q, eng in enumerate((nc.sync, nc.gpsimd) * 2):
                            eng.dma_start(
                                x_nat4[:, q : q + 1, :],
                                x_in[q * 128 : (q + 1) * 128, :]
                                .rearrange("(c p) e -> p c e", p=128)
                                .bitcast(F32R),
                            )
                    else:
                        for h, eng in ((0, nc.sync), (1, nc.gpsimd)):
                            eng.dma_start(
                                x_nat4[:, h * 2 : (h + 1) * 2, :],
                                x_in[nt * 512 + h * 256 : nt * 512 + (h + 1) * 256, :]
                                .rearrange("(c p) e -> p c e", p=128)
                                .bitcast(F32R),
                            )
                    x_nats.append(x_nat4)

                # mask tile DMAs, round-robin over the three queues, emitted
                # after x so each queue drains x first (program order)
                m_tiles = {}
                for mc in range(SC):
                    m_i32 = p_m.tile([128, S], I32)
                    eng = getattr(nc, MASK_ENG[mc % 2])
                    eng.dma_start(m_i32[:], m_in[mc * 128 : (mc + 1) * 128, :])
                    m_tiles[mc] = m_i32

                for nt in range(QB):
                    x_nat4 = x_nats[nt]
                    x_T = p_xT.tile([128, EC, 512], F32R)
                    # all transposes+copies of the block first, qkv after:
                    # keeps the in-order PE queue free of qkv instructions
                    # that would stall waiting on the copies
                    for c4 in range(4):
                        for g in range(2):
                            pst = ps_t.tile([128, 512], F32R, name="pst")
                            for j4 in range(4):
                                j = g * 4 + j4
                                nc.tensor.transpose(
                                    pst[:, j4 * 128 : (j4 + 1) * 128],
                                    x_nat4[:, c4, j * 128 : (j + 1) * 128],
                                    ident_r[:],
                                )
                            dst = x_T[
                                :, g * 4 : (g + 1) * 4,
                                c4 * 128 : (c4 + 1) * 128,
                            ]
                            src = pst[:].rearrange("p (j f) -> p j f", j=4)
                            if (c4 + g) % 2 == 0:
                                nc.vector.tensor_copy(dst, src)
                            else:
                                nc.scalar.copy(dst, src)  # ACT

                    for p2 in range(2):
                        ps1 = ps_mm.tile([128, 256], F32, name="psmm")
                        ps2 = ps_mm.tile([128, 256], F32, name="psmm")
                        cs = slice(p2 * 256, (p2 + 1) * 256)
                        gsl = slice(nt * 512 + p2 * 256, nt * 512 + (p2 + 1) * 256)
                        for j in range(EC):
                            nc.tensor.matmul(
                                ps1[:], wst1[:, j, :], x_T[:, j, cs],
                                start=(j == 0), stop=(j == EC - 1),
                            )
                        for j in range(EC):
                            nc.tensor.matmul(
                                ps2[:], wst2[:, j, :], x_T[:, j, cs],
                                start=(j == 0), stop=(j == EC - 1),
                            )
                        # kT is a pure copy (k-bias dropped: softmax-invariant)
                        nc.vector.tensor_scalar_add(
                            qv[0:64, gsl], ps1[0:64, :], b_q[:]
                        )
                        nc.scalar.copy(kT[:, gsl], ps2[0:64, :])
                        nc.vector.tensor_scalar_add(
                            qv[64:128, gsl], ps2[64:128, :], b_v128[64:128, :]
                        )

                    # v natural (+ones col) for this block's four s-chunks
                    psv = ps_mm.tile([128, 256], F32R, name="psmm")
                    for j in range(4):
                        c = nt * 4 + j
                        nc.tensor.transpose(
                            psv[:, j * 64 : (j + 1) * 64],
                            qv[64:128, c * 128 : (c + 1) * 128],
                            ident_r[64:128, 64:128],
                        )
                    nc.scalar.copy(
                        v1[:, nt * 4 : (nt + 1) * 4, 0:DH],
                        psv[:, 0:256].rearrange("p (j f) -> p j f", j=4),
                    )

            # ---- phase 2: attention --------------------------------------
            with mask_ctx:
                with (
                    tc.tile_pool(name="u", bufs=6) as p_u,
                    tc.tile_pool(name="nz", bufs=3) as p_nz,
                    tc.tile_pool(name="osb", bufs=3) as p_o,
                    tc.tile_pool(name="ps_s", bufs=2, space="PSUM") as ps_s,
                    tc.tile_pool(name="ps_tm", bufs=2, space="PSUM") as ps_tm,
                    tc.tile_pool(name="ps_o", bufs=1, space="PSUM") as ps_o,
                    tc.tile_pool(name="ps_t2", bufs=1, space="PSUM") as ps_t2,
                ):
                    for bi, (q0, width) in enumerate(BLOCKS):
                        nmc = width // 128
                        mviews = [
                            m_tiles[q0 // 128 + mc][:].bitcast(BF16)
                            for mc in range(nmc)
                        ]
                        po_f = ps_o.tile([DH + 1, 512], F32, name="po")
                        po = po_f[:, :width]
                        # software-pipelined: transposes+scores of pair p
                        # are emitted before exp/cp/av of pair p-1
                        stage = []
                        for kp in range(KC // 2):
                            pstm_f = ps_tm.tile([128, 2, 512], BF16, name="pstm")
                            upair_f = p_u.tile([128, 2, 512], BF16, tag="u")
                            pss_f = ps_s.tile([128, 2, 512], F32, name="pss")
                            for j in range(2):
                                kc = kp * 2 + j
                                pstm = pstm_f[:, j, :width]
                                for mc in range(nmc):
                                    nc.tensor.matmul(
                                        pstm[:, mc * 128 : (mc + 1) * 128],
                                        mviews[mc][:, 2 * kc * 128 : 2 * (kc + 1) * 128 : 2],
                                        ident_bf[:],
                                        is_transpose=True,
                                    )
                                nc.tensor.matmul(
                                    pss_f[:, j, :width],
                                    kT[:, kc * 128 : (kc + 1) * 128],
                                    qv[0:64, q0 : q0 + width],
                                    start=True,
                                    stop=True,
                                )
                            stage.append((kp, pstm_f, upair_f, pss_f))
                            if len(stage) == 2 or kp == KC // 2 - 1:
                                for kp2, pstm2, upair2, pss2 in (
                                    list(stage) if kp == KC // 2 - 1 else stage[:1]
                                ):
                                    nc.scalar.activation(
                                        upair2[:, :, :width],
                                        pss2[:, :, :width], AF.Exp,
                                    )
                                    nc.vector.copy_predicated(
                                        upair2[:, :, :width],
                                        pstm2[:, :, :width].bitcast(I16),
                                        ones_bf[:, :, :width],
                                    )
                                    for j in range(2):
                                        kc = kp2 * 2 + j
                                        nc.tensor.matmul(
                                            po[:],
                                            v1[:, kc, :],
                                            upair2[:, j, :width],
                                            start=(kc == 0),
                                            stop=(kc == KC - 1),
                                        )
                                    stage.remove((kp2, pstm2, upair2, pss2))

                        numz_f = p_nz.tile([DH + 1, 512], F32, tag="nz")
                        numz = numz_f[:, :width]
                        if bi % 2 == 0:
                            nc.vector.tensor_copy(numz[:], po[:])
                        else:
                            nc.scalar.copy(numz[:], po[:])
                        o_sbn_f = p_o.tile([128, 4, DH], F32, tag="osb")
                        o_sbn = o_sbn_f[:, :nmc, :]
                        pt2 = ps_t2.tile([128, 4, DH + 1], F32, name="pt2")
                        for i in range(nmc):
                            nc.tensor.transpose(
                                pt2[:, i, :],
                                numz[:, i * 128 : (i + 1) * 128],
                                ident[0 : DH + 1, 0 : DH + 1],
                            )
                        r_cols = p_o.tile([128, 4], F32, tag="rcol")
                        nc.vector.reciprocal(
                            r_cols[:, 0:nmc], pt2[:, 0:nmc, DH]
                        )
                        for i in range(nmc):
                            if i % 2 == 0:
                                nc.vector.tensor_scalar_mul(
                                    o_sbn[:, i, :], pt2[:, i, 0:DH],
                                    r_cols[:, i : i + 1],
                                )
                            else:
                                nc.scalar.activation(
                                    o_sbn[:, i, :], pt2[:, i, 0:DH], AF.Copy,
                                    scale=r_cols[:, i : i + 1],
                                )
                        out_eng = nc.sync if bi == len(BLOCKS) - 1 else nc.gpsimd
                        out_eng.dma_start(
                            o_out[q0 : q0 + width, :].rearrange(
                                "(i p) d -> p i d", p=128
                            ),
                            o_sbn[:],
                        )

    nc.finalize()
    return nc


_CACHED_NC = None


def _get_nc():
    global _CACHED_NC
    if _CACHED_NC is None:
        _CACHED_NC = build(bacc.Bacc())
    return _CACHED_NC


def kernel(x, mask, W, b, _trace=False, _tmpdir=None):
    """Full inputs in, full output out. Shards batch across 8 neuron cores."""
    x = np.ascontiguousarray(x, dtype=np.float32)
    mask = np.ascontiguousarray(mask, dtype=np.int32)
    W = np.ascontiguousarray(W, dtype=np.float32)
    b = np.ascontiguousarray(b, dtype=np.float32)
    assert x.shape == (B, S, E) and mask.shape == (B, S, S)

    nc = _get_nc()
    in_maps = [
        {"x": x[c], "mask": mask[c], "W": W, "b": b} for c in range(N_CORES)
    ]
    res = run_bass_kernel_spmd(
        nc, in_maps, list(range(N_CORES)), trace=_trace, tmpdir=_tmpdir
    )
    out = np.stack([res.results[c]["out"] for c in range(N_CORES)])
    if _trace:
        return out, res
    return out
